# revision 18
# baseline (speedup 1.0000x reference)
"""Bahdanau-style attention kernel for Trainium2, data-parallel over 8 NeuronCores.

reference math (per batch row b):
    dec    = decoder_hidden @ W1^T                      [B, H]
    enc    = encoder_outputs @ W2^T                     [B, S, H]
    energy = tanh(dec[:,None,:] + enc) @ v[0]           [B, S]
    attn   = softmax(energy, axis=1)                    [B, S]
    context= attn @ encoder_outputs  (batched)          [B, H]
    returns (context, attn)

Device strategy (per core, B_local = 16):
  - Host pre-shards batch over 8 cores and supplies encoder_outputs twice in
    bf16, both pre-tiled to SBUF layout so every DMA descriptor is one fat
    contiguous row per partition: transposed [p, k, s] for the enc matmul
    (contracts h on partitions) and natural-tiled [p, t, o] for the context
    accumulation (contracts s on partitions). This avoids on-chip transposes,
    which have no cheap path for f32/bf16 at this volume.
  - enc^T[o,t] accumulated in PSUM via W2T-stationary matmuls; ACT applies
    tanh with per-partition bias = dec^T[o,b] straight out of PSUM (bf16 out,
    [128,1024] per instruction to amortize the 352-cycle ACT overhead).
  - energy^T[t,1] via tanh-stationary matmuls against v -> PSUM [128, 16]
    per batch; one ACT Exp (f32, accum_out gives free-dim partial sums).
  - softmax runs without max subtraction: |energy| <= ||v||_1 ~ 13, exp is
    comfortably inside f32 range.
  - Z via GPSIMD partition_all_reduce of the Exp accumulator; DVE reciprocal.
  - context^T[o,1] via E-tile-stationary matmuls against p (bf16).
All matmuls are bf16 (f32 matmuls fold LDWEIGHTS into the Matmult, which can
then carry only one semaphore wait - walrus rejects multi-wait fused matmuls).
Outputs are written in DMA-friendly tiled layouts and unshuffled on host.
"""

import sys

if "/opt/trn_rl_repo" not in sys.path:
    sys.path.insert(0, "/opt/trn_rl_repo")

import functools

import numpy as np

H = 256
S = 2048
B = 128
NCORES = 8
BL = B // NCORES  # 16 batches per core
NT = S // 128  # 16 token tiles per batch
# packed constant columns: w2t [0:256], w1t [256:512], dht [512:528], v [528]
CW2, CW1, CDH, CV = 0, H, 2 * H, 2 * H + BL
NC_ = CV + 1  # 529


@functools.lru_cache(maxsize=1)
def _build():
    import concourse.bacc as bacc
    import concourse.bass_isa as bass_isa
    import concourse.mybir as mybir
    from concourse.tile import TileContext

    f32 = mybir.dt.float32
    bf16 = mybir.dt.bfloat16
    Tanh = mybir.ActivationFunctionType.Tanh
    Exp = mybir.ActivationFunctionType.Exp

    nc = bacc.Bacc()

    # pre-tiled on host: et[b, p, k, s] = E[b, s, k*128+p]
    et = nc.declare_dram_parameter("et", [BL, 128, 2 * S], bf16, isOutput=False)
    # pre-tiled on host: en[b, p, t, o] = E[b, t*128+p, o]
    en = nc.declare_dram_parameter("en", [BL, 128, NT * H], bf16, isOutput=False)
    consts = nc.declare_dram_parameter("consts", [H, NC_], bf16, isOutput=False)
    # attn_o[t, b*NT + tile] = p(b, tile*128 + t) / Z_b
    attn_o = nc.declare_dram_parameter("attn_o", [128, BL * NT], f32, isOutput=True)
    # ctx_o[o, b*2 + m] = context(b, m*128 + o)
    ctx_o = nc.declare_dram_parameter("ctx_o", [128, BL * 2], f32, isOutput=True)

    with TileContext(nc) as tc:
        with (
            tc.tile_pool(name="const", bufs=1) as const,
            tc.tile_pool(name="io", bufs=1) as io,
            tc.tile_pool(name="ein", bufs=5) as ein,
            tc.tile_pool(name="work", bufs=6) as work,
            tc.tile_pool(name="small", bufs=3) as small,
            tc.tile_pool(name="ps_enc", bufs=3, space="PSUM") as ps_enc,
            tc.tile_pool(name="ps_eT", bufs=1, space="PSUM") as ps_eT,
            tc.tile_pool(name="ps_small", bufs=1, space="PSUM") as ps_small,
        ):
            # --- constants: one DMA, one semaphore -------------------------
            c_sb = const.tile([128, 2, NC_], bf16)
            nc.sync.dma_start(out=c_sb, in_=consts.rearrange("(k p) c -> p k c", p=128))

            def w2t_sb(k, m):
                return c_sb[:, k, CW2 + m * 128 : CW2 + (m + 1) * 128]

            def w1t_sb(k, m):
                return c_sb[:, k, CW1 + m * 128 : CW1 + (m + 1) * 128]

            dht_sb = c_sb[:, :, CDH : CDH + BL]
            v_sb = c_sb[:, :, CV : CV + 1]

            # --- output staging -------------------------------------------
            attn_sb = io.tile([128, BL * NT], f32)
            ctx_sb = io.tile([128, BL * 2], f32)

            # --- dec^T = W1^T-contraction of decoder_hidden ----------------
            decT_sb = const.tile([128, 2, BL], f32)
            for m in range(2):
                dec_ps = ps_small.tile([128, BL], f32, tag="ctx")
                for k in range(2):
                    nc.tensor.matmul(
                        dec_ps,
                        w1t_sb(k, m),
                        dht_sb[:, k, :],
                        start=(k == 0),
                        stop=(k == 1),
                    )
                nc.vector.tensor_copy(decT_sb[:, m, :], dec_ps)

            # --- main loop over local batches ------------------------------
            # Deep software pipeline. Per segment b the PE program order is
            #   A(b)=enc h0 | E1(b-1)=energy h1 | X(b-1)=exp+Z (no PE) |
            #   B(b)=enc h1 | E0(b)=energy h0 | C(b-1)=ctx+scales
            # so every energy/ctx group has >2.3us of enc work between it and
            # the tanh/exp producing its operands (the PE runs in program
            # order; this spacing is what keeps it from stalling).
            st = {}  # per-b carried tiles

            def emit_enc(b, half):
                etb = st[b]["etb"]
                tanh_h = []
                for m in range(2):
                    enc_ps = ps_enc.tile([128, 1024], f32, tag="enc")
                    for k in range(2):
                        for c in range(2):
                            nc.tensor.matmul(
                                enc_ps[:, c * 512 : (c + 1) * 512],
                                w2t_sb(k, m),
                                etb[:, k, half * 1024 + c * 512 :][:, :512],
                                start=(k == 0),
                                stop=(k == 1),
                            )
                    th = work.tile([128, 1024], bf16, tag="tanh")
                    nc.scalar.activation(
                        out=th,
                        in_=enc_ps,
                        func=Tanh,
                        bias=decT_sb[:, m, b : b + 1],
                        scale=1.0,
                    )
                    tanh_h.append(th)
                st[b][f"tanh{half}"] = tanh_h

            def emit_energy(b, half):
                tanh_h = st[b].pop(f"tanh{half}")
                eT_ps = st[b]["eT_ps"]
                for ti in range(8):
                    gt = half * 8 + ti
                    for m in range(2):
                        nc.tensor.matmul(
                            eT_ps[:, gt : gt + 1],
                            tanh_h[m][:, ti * 128 : (ti + 1) * 128],
                            v_sb[:, m, :],
                            start=(m == 0),
                            stop=(m == 1),
                        )

            def emit_x(b):
                eT_ps = st[b].pop("eT_ps")
                pT_f32 = small.tile([128, NT], f32, tag="pf")
                zrow = small.tile([128, 1], f32, tag="zrow")
                nc.scalar.activation(out=pT_f32, in_=eT_ps, func=Exp, accum_out=zrow)
                pT_bf = small.tile([128, NT], bf16, tag="pb")
                nc.vector.tensor_copy(pT_bf, pT_f32)
                z128 = small.tile([128, 1], f32, tag="z128")
                nc.gpsimd.partition_all_reduce(
                    z128, zrow, channels=128, reduce_op=bass_isa.ReduceOp.add
                )
                rz_sb = small.tile([128, 1], f32, tag="rz")
                nc.vector.reciprocal(rz_sb, z128)
                st[b].update(pT_f32=pT_f32, pT_bf=pT_bf, rz=rz_sb)

            def emit_ctx(b):
                s = st.pop(b)
                enb, pT_bf, pT_f32, rz_sb = s["enb"], s["pT_bf"], s["pT_f32"], s["rz"]
                ctx_ps = ps_small.tile([128, 2], f32, tag="ctx")
                for m in range(2):
                    for ti in range(NT):
                        nc.tensor.matmul(
                            ctx_ps[:, m : m + 1],
                            enb[:, ti, m * 128 : (m + 1) * 128],
                            pT_bf[:, ti : ti + 1],
                            start=(ti == 0),
                            stop=(ti == NT - 1),
                        )
                nc.vector.tensor_scalar_mul(ctx_sb[:, b * 2 : (b + 1) * 2], ctx_ps, rz_sb)
                nc.vector.tensor_scalar_mul(
                    attn_sb[:, b * NT : (b + 1) * NT], pT_f32, rz_sb
                )

            def emit_loads(b, chunks=2):
                etb = ein.tile([128, 2, S], bf16, tag="et")
                for c in range(chunks):
                    w = S // chunks
                    for k in range(2):
                        nc.sync.dma_start(
                            out=etb[:, k, c * w : (c + 1) * w],
                            in_=et[b, :, k * S + c * w : k * S + (c + 1) * w],
                        )
                enb = ein.tile([128, NT, H], bf16, tag="en")
                for hh in range(2):
                    nc.sync.dma_start(
                        out=enb[:, hh * (NT // 2) : (hh + 1) * (NT // 2), :],
                        in_=en[b, :, hh * (NT * H // 2) : (hh + 1) * (NT * H // 2)],
                    )
                eT_ps = ps_eT.tile([128, NT], f32, tag="eT")
                st[b] = {"etb": etb, "enb": enb, "eT_ps": eT_ps}

            for b in range(BL):
                if b == 0:
                    emit_loads(0)
                    emit_loads(1)
                elif b + 1 < BL:
                    emit_loads(b + 1)
                emit_enc(b, 0)  # A(b)
                if b > 0:
                    emit_energy(b - 1, 0)  # E0(b-1)
                emit_enc(b, 1)  # B(b)
                if b > 0:
                    emit_energy(b - 1, 1)  # E1(b-1)
                    emit_x(b - 1)  # X(b-1)
                if b > 1:
                    emit_ctx(b - 2)  # C(b-2)
            emit_energy(BL - 1, 0)
            emit_energy(BL - 1, 1)
            emit_x(BL - 1)
            emit_ctx(BL - 2)
            emit_ctx(BL - 1)

            nc.sync.dma_start(out=attn_o[:, :], in_=attn_sb)
            nc.sync.dma_start(out=ctx_o[:, :], in_=ctx_sb)

    nc.compile()
    return nc


def make_in_maps(decoder_hidden, encoder_outputs, W1, W2, v):
    import ml_dtypes

    bf16 = ml_dtypes.bfloat16
    in_maps = []
    consts0 = np.zeros((H, NC_), np.float32)
    consts0[:, CW2 : CW2 + H] = np.asarray(W2, np.float32).T
    consts0[:, CW1 : CW1 + H] = np.asarray(W1, np.float32).T
    consts0[:, CV] = np.asarray(v, np.float32).reshape(H)
    for c in range(NCORES):
        sl = slice(c * BL, (c + 1) * BL)
        E = np.asarray(encoder_outputs[sl], dtype=np.float32)
        consts = consts0.copy()
        consts[:, CDH : CDH + BL] = np.asarray(decoder_hidden[sl], np.float32).T
        # et[b, p, k, s] = E[b, s, k*128+p]
        et = E.transpose(0, 2, 1).reshape(BL, 2, 128, S).transpose(0, 2, 1, 3)
        # en[b, p, t, o] = E[b, t*128+p, o]
        en = E.reshape(BL, NT, 128, H).transpose(0, 2, 1, 3)
        in_maps.append(
            {
                "et": np.ascontiguousarray(et.reshape(BL, 128, 2 * S)).astype(bf16),
                "en": np.ascontiguousarray(en.reshape(BL, 128, NT * H)).astype(bf16),
                "consts": consts.astype(bf16),
            }
        )
    return in_maps


def postprocess(results):
    ctx_full = np.empty((B, H), np.float32)
    attn_full = np.empty((B, S), np.float32)
    for c, r in enumerate(results):
        a = np.asarray(r["attn_o"], np.float32).reshape(128, BL, NT)
        attn_full[c * BL : (c + 1) * BL] = a.transpose(1, 2, 0).reshape(BL, S)
        ctx = np.asarray(r["ctx_o"], np.float32).reshape(128, BL, 2)
        ctx_full[c * BL : (c + 1) * BL] = ctx.transpose(1, 2, 0).reshape(BL, H)
    return ctx_full, attn_full


def kernel(decoder_hidden, encoder_outputs, W1, W2, v):
    from concourse.bass_utils import run_bass_kernel_spmd

    nc = _build()
    in_maps = make_in_maps(decoder_hidden, encoder_outputs, W1, W2, v)
    res = run_bass_kernel_spmd(nc, in_maps, core_ids=list(range(NCORES)))
    return postprocess(res.results)


# revision 19
# speedup vs baseline: 1.0374x; 1.0374x over previous
"""Bahdanau-style attention kernel for Trainium2, data-parallel over 8 NeuronCores.

reference math (per batch row b):
    dec    = decoder_hidden @ W1^T                      [B, H]
    enc    = encoder_outputs @ W2^T                     [B, S, H]
    energy = tanh(dec[:,None,:] + enc) @ v[0]           [B, S]
    attn   = softmax(energy, axis=1)                    [B, S]
    context= attn @ encoder_outputs  (batched)          [B, H]
    returns (context, attn)

Device strategy (per core, B_local = 16):
  - Host pre-shards batch over 8 cores and supplies encoder_outputs twice in
    bf16, both pre-tiled to SBUF layout so every DMA descriptor is one fat
    contiguous row per partition: transposed [p, k, s] for the enc matmul
    (contracts h on partitions) and natural-tiled [p, t, o] for the context
    accumulation (contracts s on partitions). This avoids on-chip transposes,
    which have no cheap path for f32/bf16 at this volume.
  - enc^T[o,t] accumulated in PSUM via W2T-stationary matmuls; ACT applies
    tanh with per-partition bias = dec^T[o,b] straight out of PSUM (bf16 out,
    [128,1024] per instruction to amortize the 352-cycle ACT overhead).
  - energy^T[t,1] via tanh-stationary matmuls against v -> PSUM [128, 16]
    per batch; one ACT Exp (f32, accum_out gives free-dim partial sums).
  - softmax runs without max subtraction: |energy| <= ||v||_1 ~ 13, exp is
    comfortably inside f32 range.
  - Z via GPSIMD partition_all_reduce of the Exp accumulator; DVE reciprocal.
  - context^T[o,1] via E-tile-stationary matmuls against p (bf16).
All matmuls are bf16 (f32 matmuls fold LDWEIGHTS into the Matmult, which can
then carry only one semaphore wait - walrus rejects multi-wait fused matmuls).
Outputs are written in DMA-friendly tiled layouts and unshuffled on host.
"""

import sys

if "/opt/trn_rl_repo" not in sys.path:
    sys.path.insert(0, "/opt/trn_rl_repo")

import functools

import numpy as np

H = 256
S = 2048
B = 128
NCORES = 8
BL = B // NCORES  # 16 batches per core
NT = S // 128  # 16 token tiles per batch
# packed constant columns: w2t [0:256], w1t [256:512], dht [512:528], v [528]
CW2, CW1, CDH, CV = 0, H, 2 * H, 2 * H + BL
NC_ = CV + 1  # 529


@functools.lru_cache(maxsize=1)
def _build():
    import concourse.bacc as bacc
    import concourse.bass_isa as bass_isa
    import concourse.mybir as mybir
    from concourse.tile import TileContext

    f32 = mybir.dt.float32
    bf16 = mybir.dt.bfloat16
    Tanh = mybir.ActivationFunctionType.Tanh
    Exp = mybir.ActivationFunctionType.Exp

    nc = bacc.Bacc()

    # pre-tiled on host: et[b, p, k, s] = E[b, s, k*128+p]
    et = nc.declare_dram_parameter("et", [BL, 128, 2 * S], bf16, isOutput=False)
    # pre-tiled on host: en[b, p, t, o] = E[b, t*128+p, o]
    en = nc.declare_dram_parameter("en", [BL, 128, NT * H], bf16, isOutput=False)
    consts = nc.declare_dram_parameter("consts", [H, NC_], bf16, isOutput=False)
    # attn_o[t, b*NT + tile] = p(b, tile*128 + t) / Z_b
    attn_o = nc.declare_dram_parameter("attn_o", [128, BL * NT], f32, isOutput=True)
    # ctx_o[o, b*2 + m] = context(b, m*128 + o)
    ctx_o = nc.declare_dram_parameter("ctx_o", [128, BL * 2], f32, isOutput=True)

    with TileContext(nc) as tc:
        with (
            tc.tile_pool(name="const", bufs=1) as const,
            tc.tile_pool(name="io", bufs=1) as io,
            tc.tile_pool(name="ein", bufs=5) as ein,
            tc.tile_pool(name="work", bufs=6) as work,
            tc.tile_pool(name="small", bufs=3) as small,
            tc.tile_pool(name="ps_enc", bufs=3, space="PSUM") as ps_enc,
            tc.tile_pool(name="ps_eT", bufs=1, space="PSUM") as ps_eT,
            tc.tile_pool(name="ps_small", bufs=1, space="PSUM") as ps_small,
        ):
            # --- constants: one DMA, one semaphore -------------------------
            c_sb = const.tile([128, 2, NC_], bf16)
            nc.sync.dma_start(out=c_sb, in_=consts.rearrange("(k p) c -> p k c", p=128))

            def w2t_sb(k, m):
                return c_sb[:, k, CW2 + m * 128 : CW2 + (m + 1) * 128]

            def w1t_sb(k, m):
                return c_sb[:, k, CW1 + m * 128 : CW1 + (m + 1) * 128]

            dht_sb = c_sb[:, :, CDH : CDH + BL]
            v_sb = c_sb[:, :, CV : CV + 1]

            # --- output staging -------------------------------------------
            attn_sb = io.tile([128, BL * NT], f32)
            ctx_sb = io.tile([128, BL * 2], f32)

            # --- dec^T = W1^T-contraction of decoder_hidden ----------------
            decT_sb = const.tile([128, 2, BL], f32)
            for m in range(2):
                dec_ps = ps_small.tile([128, BL], f32, tag="ctx")
                for k in range(2):
                    nc.tensor.matmul(
                        dec_ps,
                        w1t_sb(k, m),
                        dht_sb[:, k, :],
                        start=(k == 0),
                        stop=(k == 1),
                    )
                nc.vector.tensor_copy(decT_sb[:, m, :], dec_ps)

            # --- main loop over local batches ------------------------------
            # Deep software pipeline. Per segment b the PE program order is
            #   A(b)=enc h0 | E1(b-1)=energy h1 | X(b-1)=exp+Z (no PE) |
            #   B(b)=enc h1 | E0(b)=energy h0 | C(b-1)=ctx+scales
            # so every energy/ctx group has >2.3us of enc work between it and
            # the tanh/exp producing its operands (the PE runs in program
            # order; this spacing is what keeps it from stalling).
            st = {}  # per-b carried tiles

            def emit_enc(b, half):
                etb = st[b]["etb"]
                tanh_h = []
                for m in range(2):
                    enc_ps = ps_enc.tile([128, 1024], f32, tag="enc")
                    for k in range(2):
                        for c in range(2):
                            nc.tensor.matmul(
                                enc_ps[:, c * 512 : (c + 1) * 512],
                                w2t_sb(k, m),
                                etb[:, k, half * 1024 + c * 512 :][:, :512],
                                start=(k == 0),
                                stop=(k == 1),
                            )
                    th = work.tile([128, 1024], bf16, tag="tanh")
                    nc.scalar.activation(
                        out=th,
                        in_=enc_ps,
                        func=Tanh,
                        bias=decT_sb[:, m, b : b + 1],
                        scale=1.0,
                    )
                    tanh_h.append(th)
                st[b][f"tanh{half}"] = tanh_h

            def emit_energy(b, half):
                tanh_h = st[b].pop(f"tanh{half}")
                eT_ps = st[b]["eT_ps"]
                for ti in range(8):
                    gt = half * 8 + ti
                    for m in range(2):
                        nc.tensor.matmul(
                            eT_ps[:, gt : gt + 1],
                            tanh_h[m][:, ti * 128 : (ti + 1) * 128],
                            v_sb[:, m, :],
                            start=(m == 0),
                            stop=(m == 1),
                        )

            def emit_x(b):
                eT_ps = st[b].pop("eT_ps")
                pT_f32 = small.tile([128, NT], f32, tag="pf")
                zrow = small.tile([128, 1], f32, tag="zrow")
                nc.scalar.activation(out=pT_f32, in_=eT_ps, func=Exp, accum_out=zrow)
                pT_bf = small.tile([128, NT], bf16, tag="pb")
                nc.vector.tensor_copy(pT_bf, pT_f32)
                z128 = small.tile([128, 1], f32, tag="z128")
                nc.gpsimd.partition_all_reduce(
                    z128, zrow, channels=128, reduce_op=bass_isa.ReduceOp.add
                )
                rz_sb = small.tile([128, 1], f32, tag="rz")
                nc.vector.reciprocal(rz_sb, z128)
                st[b].update(pT_f32=pT_f32, pT_bf=pT_bf, rz=rz_sb)

            def emit_ctx(b):
                s = st.pop(b)
                enb, pT_bf, pT_f32, rz_sb = s["enb"], s["pT_bf"], s["pT_f32"], s["rz"]
                ctx_ps = ps_small.tile([128, 2], f32, tag="ctx")
                for m in range(2):
                    for ti in range(NT):
                        nc.tensor.matmul(
                            ctx_ps[:, m : m + 1],
                            enb[:, ti, m * 128 : (m + 1) * 128],
                            pT_bf[:, ti : ti + 1],
                            start=(ti == 0),
                            stop=(ti == NT - 1),
                        )
                nc.vector.tensor_scalar_mul(ctx_sb[:, b * 2 : (b + 1) * 2], ctx_ps, rz_sb)
                nc.vector.tensor_scalar_mul(
                    attn_sb[:, b * NT : (b + 1) * NT], pT_f32, rz_sb
                )

            def emit_loads(b):
                etb = ein.tile([128, 2, S], bf16, tag="et")
                for k in range(2):
                    nc.sync.dma_start(out=etb[:, k, :], in_=et[b, :, k * S : (k + 1) * S])
                enb = ein.tile([128, NT, H], bf16, tag="en")
                for hh in range(2):
                    nc.sync.dma_start(
                        out=enb[:, hh * (NT // 2) : (hh + 1) * (NT // 2), :],
                        in_=en[b, :, hh * (NT * H // 2) : (hh + 1) * (NT * H // 2)],
                    )
                eT_ps = ps_eT.tile([128, NT], f32, tag="eT")
                st[b] = {"etb": etb, "enb": enb, "eT_ps": eT_ps}

            for b in range(BL):
                if b == 0:
                    emit_loads(0)
                    emit_loads(1)
                elif b + 1 < BL:
                    emit_loads(b + 1)
                emit_enc(b, 0)  # A(b)
                if b > 0:
                    emit_energy(b - 1, 0)  # E0(b-1)
                emit_enc(b, 1)  # B(b)
                if b > 0:
                    emit_energy(b - 1, 1)  # E1(b-1)
                    emit_x(b - 1)  # X(b-1)
                if b > 1:
                    emit_ctx(b - 2)  # C(b-2)
            emit_energy(BL - 1, 0)
            emit_energy(BL - 1, 1)
            emit_x(BL - 1)
            emit_ctx(BL - 2)
            emit_ctx(BL - 1)

            nc.sync.dma_start(out=attn_o[:, :], in_=attn_sb)
            nc.sync.dma_start(out=ctx_o[:, :], in_=ctx_sb)

    nc.compile()
    return nc


def make_in_maps(decoder_hidden, encoder_outputs, W1, W2, v):
    import ml_dtypes

    bf16 = ml_dtypes.bfloat16
    in_maps = []
    consts0 = np.zeros((H, NC_), np.float32)
    consts0[:, CW2 : CW2 + H] = np.asarray(W2, np.float32).T
    consts0[:, CW1 : CW1 + H] = np.asarray(W1, np.float32).T
    consts0[:, CV] = np.asarray(v, np.float32).reshape(H)
    for c in range(NCORES):
        sl = slice(c * BL, (c + 1) * BL)
        E = np.asarray(encoder_outputs[sl], dtype=np.float32)
        consts = consts0.copy()
        consts[:, CDH : CDH + BL] = np.asarray(decoder_hidden[sl], np.float32).T
        # et[b, p, k, s] = E[b, s, k*128+p]
        et = E.transpose(0, 2, 1).reshape(BL, 2, 128, S).transpose(0, 2, 1, 3)
        # en[b, p, t, o] = E[b, t*128+p, o]
        en = E.reshape(BL, NT, 128, H).transpose(0, 2, 1, 3)
        in_maps.append(
            {
                "et": np.ascontiguousarray(et.reshape(BL, 128, 2 * S)).astype(bf16),
                "en": np.ascontiguousarray(en.reshape(BL, 128, NT * H)).astype(bf16),
                "consts": consts.astype(bf16),
            }
        )
    return in_maps


def postprocess(results):
    ctx_full = np.empty((B, H), np.float32)
    attn_full = np.empty((B, S), np.float32)
    for c, r in enumerate(results):
        a = np.asarray(r["attn_o"], np.float32).reshape(128, BL, NT)
        attn_full[c * BL : (c + 1) * BL] = a.transpose(1, 2, 0).reshape(BL, S)
        ctx = np.asarray(r["ctx_o"], np.float32).reshape(128, BL, 2)
        ctx_full[c * BL : (c + 1) * BL] = ctx.transpose(1, 2, 0).reshape(BL, H)
    return ctx_full, attn_full


def kernel(decoder_hidden, encoder_outputs, W1, W2, v):
    from concourse.bass_utils import run_bass_kernel_spmd

    nc = _build()
    in_maps = make_in_maps(decoder_hidden, encoder_outputs, W1, W2, v)
    res = run_bass_kernel_spmd(nc, in_maps, core_ids=list(range(NCORES)))
    return postprocess(res.results)
